# revision 2
# baseline (speedup 1.0000x reference)
"""Trainium2 Bass kernel v2 for 3-layer EGAT message passing (nn_COUNTYOD).

Single-pass-per-layer design, bf16 storage, 8 cores, edge parallelism by
dst range (device d owns dst nodes [d*6272, (d+1)*6272)).

Per layer:
  - Node tables cat[n] = [nh@Wni + b | nh@Wns | pad64] as bf16 256-col rows
    (512B -> full-rate DMA gather descriptors). Layer 0: computed
    redundantly on every device from host-replicated nh0 (no collective).
    Layers 1-2: own 49-block slice computed in the previous layer's block
    epilogue, then one AllGather.
  - nj handled per-block: njtab[b] = nh_blk @ Wnj kept in SBUF; per-edge
    nj_e = ohT.T @ njtab via the transposed scatter one-hot (PE).
  - Single edge pass per block: batched indirect gather of [ni|ns] rows by
    src, f = leaky(fij + ni + nj), e-logit via DVE mul+grouped-reduce,
    ex = exp(e - 30) (constant shift cancels in the softmax ratio),
    msg = ex * ns, one-hot scatter-matmul into PSUM accumulating
    [msg | ex]; h = relu?(acc)/s.
  - relu(f) stored bf16 transposed ([96, E'] via PE transpose) for the next
    layer's fij matmul lhsT.
"""

import sys
import numpy as np

for _p in ("/opt/trn_rl_repo",):
    if _p not in sys.path:
        sys.path.insert(0, _p)

import ml_dtypes
import concourse.bass as bass  # noqa: E402
import concourse.bacc as bacc  # noqa: E402
import concourse.mybir as mybir  # noqa: E402
import concourse.tile as tile  # noqa: E402
from concourse.bass import IndirectOffsetOnAxis  # noqa: E402
from concourse.masks import make_identity  # noqa: E402

F32 = mybir.dt.float32
BF16 = mybir.dt.bfloat16
I32 = mybir.dt.int32
AF = mybir.ActivationFunctionType
ALU = mybir.AluOpType
bfnp = ml_dtypes.bfloat16

P = 128
FD = 96
CW = 256          # cat table row width (bf16 -> 512B descriptors)
H = 3
HE = 32
EPS = 1e-30
EBIAS = -30.0     # ex = exp(e + EBIAS); cancels in softmax ratio


HALF = 32768


class Cfg:
    def __init__(self, ndev, nblk, TL, TH, odf, n_real, g=4):
        self.ndev = ndev
        self.nblk = nblk
        self.TL = [int(x) for x in TL]   # per-block low-half tile counts
        self.TH = [int(x) for x in TH]   # per-block high-half tile counts
        self.Tb = [a + b for a, b in zip(TL, TH)]
        self.offs = np.concatenate([[0], np.cumsum(self.Tb)]).astype(int)
        self.Tmax = max(self.Tb)
        self.odf = odf
        self.n_real = n_real
        self.g = g
        self.R = nblk * P
        self.npad = ndev * self.R
        self.nblk_all = ndev * nblk
        self.ttot = int(self.offs[-1])
        self.epd = self.ttot * P


def host_prep(inputs, cfg):
    src = np.asarray(inputs["src"]).astype(np.int64)
    dst = np.asarray(inputs["dst"]).astype(np.int64)

    order = np.argsort(dst, kind="stable")
    ssrc, sdst = src[order], dst[order]
    ef0 = np.asarray(inputs["countyodfeats"]).astype(np.float32)[order]

    nb_all = cfg.nblk_all
    blkcnt = np.bincount(sdst // P, minlength=nb_all)
    starts = np.zeros(nb_all + 1, np.int64)
    np.cumsum(blkcnt, out=starts[1:])

    percore = []
    for d in range(cfg.ndev):
        dloc = np.full((cfg.ttot * P,), -1.0, np.float32)
        efp = np.zeros((cfg.ttot * P, cfg.odf), np.float32)
        idx16 = np.zeros((cfg.ttot * P,), np.int16)
        for b in range(cfg.nblk):
            gblk = d * cfg.nblk + b
            s0, s1 = starts[gblk], starts[gblk + 1]
            es, ed, ee = ssrc[s0:s1], sdst[s0:s1], ef0[s0:s1]
            lo = es < HALF
            base = cfg.offs[b] * P
            nlo = int(lo.sum())
            nhi = len(es) - nlo
            assert nlo <= cfg.TL[b] * P and nhi <= cfg.TH[b] * P, (nlo, nhi)
            bhi = base + cfg.TL[b] * P
            dloc[base : base + nlo] = (ed[lo] - gblk * P).astype(np.float32)
            efp[base : base + nlo] = ee[lo]
            idx16[base : base + nlo] = es[lo].astype(np.int16)
            dloc[bhi : bhi + nhi] = (ed[~lo] - gblk * P).astype(np.float32)
            efp[bhi : bhi + nhi] = ee[~lo]
            idx16[bhi : bhi + nhi] = (es[~lo] - HALF).astype(np.int16)
        # [16, n/16] wrapped, replicated across the 8 gpsimd cores (128 part)
        iwrap = np.tile(idx16.reshape(-1, 16).T, (8, 1))
        percore.append(
            dict(
                idx16=np.ascontiguousarray(iwrap),
                dstloc=np.ascontiguousarray(
                    dloc.reshape(-1, P).T.astype(np.float32)
                ),
                ef0T=np.ascontiguousarray(efp.T.astype(bfnp)),
            )
        )

    nh0 = np.asarray(inputs["ndata_h"]).astype(np.float32)
    nh0p = np.zeros((cfg.npad, FD), np.float32)
    nh0p[: cfg.n_real] = nh0[: cfg.n_real]
    nh0f = np.ones((FD + 1, cfg.npad), np.float32)
    nh0f[:FD] = nh0p.T
    nh0f = np.ascontiguousarray(nh0f.astype(bfnp))
    for d in range(cfg.ndev):
        percore[d]["nh0f"] = nh0f
        sl = np.ones((FD + 1, cfg.R), np.float32)
        sl[:FD] = nh0p[d * cfg.R : (d + 1) * cfg.R].T
        percore[d]["nh0T"] = np.ascontiguousarray(sl.astype(bfnp))

    def wcat(Wni, Wns, bias, attnv):
        # row layout: [ni 96 | e_ni 3 | ns 96 | pad], bias folded into row 96
        w = np.zeros((FD + 1, CW), np.float32)
        w[:FD, 0:FD] = Wni
        w[FD, 0:FD] = bias
        w[:FD, FD : FD + H] = Wni @ attnv.T
        w[FD, FD : FD + H] = bias @ attnv.T
        w[:FD, FD + H : 2 * FD + H] = Wns
        return np.ascontiguousarray(w.astype(bfnp))

    def wnj(Wnj, attnv):
        w = np.zeros((FD + 1, FD + H), np.float32)
        w[:FD, 0:FD] = Wnj
        w[:FD, FD : FD + H] = Wnj @ attnv.T
        return np.ascontiguousarray(w.astype(bfnp))

    def wfij_ext(Wf, attnv):
        w = np.concatenate([Wf, Wf @ attnv.T], axis=1)
        return np.ascontiguousarray(w.astype(bfnp))

    g = lambda k: np.asarray(inputs[k]).astype(np.float32)
    a0 = g("attn0").reshape(H, HE)
    atl = g("attn").reshape(2, H, HE)
    # attnv[h] spread across 96 dims: attnv[h, h*HE:(h+1)*HE] = attn[h]
    def spread(a):
        m = np.zeros((H, FD), np.float32)
        for h in range(H):
            m[h, h * HE : (h + 1) * HE] = a[h]
        return m
    av = [spread(a0), spread(atl[0]), spread(atl[1])]
    weights = {}
    weights["wcat0"] = wcat(g("Wni0"), g("Wns0"), g("bias0"), av[0])
    weights["wcat1"] = wcat(g("Wni")[0], g("Wns")[0], g("bias")[0], av[1])
    weights["wcat2"] = wcat(g("Wni")[1], g("Wns")[1], g("bias")[1], av[2])
    weights["wnj0"] = wnj(g("Wnj0"), av[0])
    weights["wnj1"] = wnj(g("Wnj")[0], av[1])
    weights["wnj2"] = wnj(g("Wnj")[1], av[2])
    weights["wfij0"] = wfij_ext(g("Wfij0"), av[0])
    weights["wfij1"] = wfij_ext(g("Wfij")[0], av[1])
    weights["wfij2"] = wfij_ext(g("Wfij")[1], av[2])
    # abc pre-scaled by 0.99 (em = fs * 0.99attn; e += 0.01 * e_lin)
    weights["abc0"] = np.ascontiguousarray(
        (0.99 * a0.reshape(1, FD)).repeat(P, 0).astype(bfnp)
    )
    at = atl.reshape(2, FD)
    weights["abc1"] = np.ascontiguousarray(
        (0.99 * at[0:1]).repeat(P, 0).astype(bfnp)
    )
    weights["abc2"] = np.ascontiguousarray(
        (0.99 * at[1:2]).repeat(P, 0).astype(bfnp)
    )
    for d in range(cfg.ndev):
        percore[d].update(weights)
    return percore


def v(t, offset, ap):
    """Strided view helper on a tile AP."""
    return bass.AP(t[:].tensor, t[:].offset + offset, ap)


def build_program(cfg, debug=False, fake_collective=False):
    nc = bacc.Bacc("TRN2", target_bir_lowering=False, debug=False,
                   dynamic_dma_scratch_size=32768)
    c = cfg
    G, Ttot, EPD = c.g, c.ttot, c.epd
    I16 = mybir.dt.int16

    pr = {}
    pr["nh0f"] = nc.declare_dram_parameter("nh0f", [FD + 1, c.npad], BF16, isOutput=False)
    pr["nh0T"] = nc.declare_dram_parameter("nh0T", [FD + 1, c.R], BF16, isOutput=False)
    pr["ef0T"] = nc.declare_dram_parameter("ef0T", [c.odf, EPD], BF16, isOutput=False)
    pr["idx16"] = nc.declare_dram_parameter("idx16", [P, Ttot * 8], I16, isOutput=False)
    pr["dstloc"] = nc.declare_dram_parameter("dstloc", [P, Ttot], F32, isOutput=False)
    for i in range(3):
        pr[f"wcat{i}"] = nc.declare_dram_parameter(f"wcat{i}", [FD + 1, CW], BF16, isOutput=False)
        pr[f"wnj{i}"] = nc.declare_dram_parameter(f"wnj{i}", [FD + 1, FD + H], BF16, isOutput=False)
        pr[f"abc{i}"] = nc.declare_dram_parameter(f"abc{i}", [P, FD], BF16, isOutput=False)
        cdim = c.odf if i == 0 else FD
        pr[f"wfij{i}"] = nc.declare_dram_parameter(f"wfij{i}", [cdim, FD + H], BF16, isOutput=False)
    out3 = nc.declare_dram_parameter("out3", [c.R, FD], F32, isOutput=True)
    dbg = {}
    if debug:
        dbg["d_cat"] = nc.declare_dram_parameter("d_cat", [c.npad, CW], F32, isOutput=True)
        dbg["d_nh"] = nc.declare_dram_parameter("d_nh", [FD + 1, c.R], F32, isOutput=True)
        dbg["d_ef"] = nc.declare_dram_parameter("d_ef", [FD, EPD], F32, isOutput=True)

    efA = nc.dram_tensor("efA", [FD, EPD], BF16)
    efB = nc.dram_tensor("efB", [FD, EPD], BF16)
    catL = nc.dram_tensor("catL", [c.R, CW], BF16)
    catG = nc.dram_tensor("catG", [c.npad, CW], BF16, addr_space="Shared")

    rg = [list(range(c.ndev))]

    with tile.TileContext(nc) as tc:
        with tc.tile_pool(name="persist", bufs=1) as pp:
            ident = pp.tile([P, P], BF16, tag="ident")
            make_identity(nc, ident[:])
            iota_i = pp.tile([P, P], I32, tag="iota_i")
            nc.gpsimd.iota(iota_i[:], pattern=[[1, P]], base=0, channel_multiplier=0)
            iota_bf = pp.tile([P, P], BF16, tag="iota_bf")
            nc.vector.tensor_copy(out=iota_bf[:], in_=iota_i[:])
            ebias = pp.tile([P, 1], F32, tag="ebias")
            nc.vector.memset(ebias[:], EBIAS)

            idx16 = pp.tile([P, Ttot * 8], I16, tag="idx16")
            dstloc = pp.tile([P, Ttot], F32, tag="dstloc")
            nc.sync.dma_start(out=idx16[:], in_=pr["idx16"][:])
            nc.sync.dma_start(out=dstloc[:], in_=pr["dstloc"][:])

            wcat_sb, wnj_sb, abc_sb, wfij_sb = [], [], [], []
            for l in range(3):
                wc = pp.tile([FD + 1, CW], BF16, tag=f"wcat{l}")
                nc.sync.dma_start(out=wc[:], in_=pr[f"wcat{l}"][:])
                wcat_sb.append(wc)
                wn = pp.tile([FD + 1, FD + H], BF16, tag=f"wnj{l}")
                nc.sync.dma_start(out=wn[:], in_=pr[f"wnj{l}"][:])
                wnj_sb.append(wn)
                ab = pp.tile([P, FD], BF16, tag=f"abc{l}")
                nc.sync.dma_start(out=ab[:], in_=pr[f"abc{l}"][:])
                abc_sb.append(ab)
                cdim = c.odf if l == 0 else FD
                wf = pp.tile([cdim, FD + H], BF16, tag=f"wfij{l}")
                nc.sync.dma_start(out=wf[:], in_=pr[f"wfij{l}"][:])
                wfij_sb.append(wf)

            njtab = pp.tile([P, c.nblk * (FD + H)], BF16, tag="njtab")
            nh_sb = pp.tile([FD + 1, c.R], BF16, tag="nh_sb")
            nc.sync.dma_start(out=nh_sb[:], in_=pr["nh0T"][:])

            # ---- layer-0 tables: full catG computed locally ----
            with (
                tc.tile_pool(name="t0", bufs=1) as t0p,
                tc.tile_pool(name="t0w", bufs=2) as t0w,
                tc.tile_pool(name="t0q", bufs=2, space="PSUM") as t0q,
            ):
                nh0f_sb = t0p.tile([FD + 1, c.npad], BF16, tag="nh0f")
                nc.sync.dma_start(out=nh0f_sb[:], in_=pr["nh0f"][:])
                BB = 4
                for m in range(0, c.nblk_all, BB):
                    stage = t0w.tile([P, BB * CW], BF16, tag="stage")
                    for k in range(BB):
                        pt = t0q.tile([P, CW], F32, tag="pt")
                        nc.tensor.matmul(
                            out=pt[:],
                            lhsT=nh0f_sb[:, (m + k) * P : (m + k + 1) * P],
                            rhs=wcat_sb[0][:],
                            start=True, stop=True,
                        )
                        nc.scalar.activation(
                            out=stage[:, k * CW : (k + 1) * CW], in_=pt[:],
                            func=AF.Copy,
                        )
                    outap = bass.AP(
                        catG[:].tensor, m * P * CW,
                        [[CW, P], [P * CW, BB], [1, CW]],
                    )
                    nc.sync.dma_start(out=outap, in_=stage[:])
                # njtab for own blocks from nh0T slice
                for b in range(c.nblk):
                    njp = t0q.tile([P, FD], F32, tag="njp")
                    nc.tensor.matmul(
                        out=njp[:],
                        lhsT=nh_sb[:, b * P : (b + 1) * P],
                        rhs=wnj_sb[0][:],
                        start=True, stop=True,
                    )
                    nc.scalar.activation(
                        out=njtab[:, b * FD : (b + 1) * FD], in_=njp[:],
                        func=AF.Copy,
                    )
            if debug:
                cg = pp.tile([P, 16], F32, tag="cgdump")  # placeholder
            # ---- layers ----
            for l in range(3):
                cdim = c.odf if l == 0 else FD
                ef_src = pr["ef0T"] if l == 0 else (efA if l == 1 else efB)
                ef_dst = efA if l == 0 else efB
                wf = wfij_sb[l]
                abc = abc_sb[l]

                with (
                    tc.tile_pool(name="blk", bufs=3) as bp,
                    tc.tile_pool(name="chk", bufs=4) as wp,
                    tc.tile_pool(name="eppool", bufs=2) as ep,
                    tc.tile_pool(name="q_oht", bufs=2, space="PSUM") as q_oht,
                    tc.tile_pool(name="q_fp", bufs=2, space="PSUM") as q_fp,
                    tc.tile_pool(name="q_ps", bufs=2, space="PSUM") as q_ps,
                    tc.tile_pool(name="q_ep", bufs=1, space="PSUM") as q_ep,
                ):
                    for b in range(c.nblk):
                        efc = bp.tile([cdim, T * P], BF16, tag="efc")
                        nc.sync.dma_start(
                            out=efc[:],
                            in_=ef_src[:, b * T * P : (b + 1) * T * P],
                        )
                        ps = q_ps.tile([P, FD + H], F32, tag="ps")
                        for c0 in range(0, T, G):
                            gc = min(G, T - c0)
                            t0 = b * T + c0
                            nins = wp.tile([P, CW * G], BF16, tag="nins")
                            for j in range(gc):
                                nc.gpsimd.indirect_dma_start(
                                    out=nins[:, j * CW : (j + 1) * CW],
                                    out_offset=None,
                                    in_=catG[:, :],
                                    in_offset=IndirectOffsetOnAxis(
                                        ap=srcidx[:, t0 + j : t0 + j + 1], axis=0
                                    ),
                                    element_offset=0,
                                )
                            oh = wp.tile([P, P * G], BF16, tag="oh")
                            for j in range(gc):
                                nc.vector.tensor_scalar(
                                    out=oh[:, j * P : (j + 1) * P],
                                    in0=iota_bf[:],
                                    scalar1=dstloc[:, t0 + j : t0 + j + 1],
                                    scalar2=None,
                                    op0=ALU.is_equal,
                                )
                            trp = q_oht.tile([P, 2 * P * G], BF16, tag="trp")
                            for j in range(gc):
                                nc.tensor.transpose(
                                    out=trp[:, j * P : (j + 1) * P],
                                    in_=oh[:, j * P : (j + 1) * P],
                                    identity=ident[:],
                                )
                            ohT = wp.tile([P, P * G], BF16, tag="ohT")
                            nc.scalar.activation(
                                out=ohT[:, : P * gc], in_=trp[:, : P * gc],
                                func=AF.Copy,
                            )
                            fp = q_fp.tile([P, FD * G], F32, tag="fp")
                            for j in range(gc):
                                nc.tensor.matmul(
                                    out=fp[:, j * FD : (j + 1) * FD],
                                    lhsT=efc[:, (c0 + j) * P : (c0 + j + 1) * P],
                                    rhs=wf[:],
                                    start=True, stop=False,
                                )
                                nc.tensor.matmul(
                                    out=fp[:, j * FD : (j + 1) * FD],
                                    lhsT=ohT[:, j * P : (j + 1) * P],
                                    rhs=njtab[:, b * FD : (b + 1) * FD],
                                    start=False, stop=True,
                                )
                            t2 = wp.tile([P, FD * G], BF16, tag="t2")
                            nc.vector.tensor_tensor(
                                out=v(t2, 0, [t2[:].ap[0], [FD, gc], [1, FD]]),
                                in0=v(fp, 0, [fp[:].ap[0], [FD, gc], [1, FD]]),
                                in1=v(nins, 0, [nins[:].ap[0], [CW, gc], [1, FD]]),
                                op=ALU.add,
                            )
                            fl = wp.tile([P, FD * G], BF16, tag="fl")
                            nc.vector.scalar_tensor_tensor(
                                out=fl[:, : FD * gc],
                                in0=t2[:, : FD * gc],
                                scalar=0.01,
                                in1=t2[:, : FD * gc],
                                op0=ALU.mult, op1=ALU.max,
                            )
                            if l < 2:
                                fs = wp.tile([P, FD * G], BF16, tag="fs")
                                nc.scalar.activation(
                                    out=fs[:, : FD * gc], in_=t2[:, : FD * gc],
                                    func=AF.Relu,
                                )
                                pTp = trp[0:FD, P * G : 2 * P * G]
                                for j in range(gc):
                                    nc.tensor.transpose(
                                        out=pTp[:, j * P : (j + 1) * P],
                                        in_=fs[:, j * FD : (j + 1) * FD],
                                        identity=ident[:],
                                    )
                                fsT = wp.tile([FD, P * G], BF16, tag="fsT")
                                nc.vector.tensor_copy(
                                    out=fsT[:, : P * gc], in_=pTp[:, : P * gc]
                                )
                                nc.sync.dma_start(
                                    out=ef_dst[:, t0 * P : (t0 + gc) * P],
                                    in_=fsT[:, : P * gc],
                                )
                            em = wp.tile([P, FD * G], BF16, tag="em")
                            nc.vector.tensor_tensor(
                                out=v(em, 0, [em[:].ap[0], [FD, gc], [1, FD]]),
                                in0=v(fl, 0, [fl[:].ap[0], [FD, gc], [1, FD]]),
                                in1=v(abc, 0, [abc[:].ap[0], [0, gc], [1, FD]]),
                                op=ALU.mult,
                            )
                            e_t = wp.tile([P, H * G], F32, tag="e_t")
                            nc.vector.tensor_reduce(
                                out=v(e_t, 0, [e_t[:].ap[0], [H, gc], [1, H]]),
                                in_=v(em, 0, [em[:].ap[0], [FD, gc], [HE, H], [1, HE]]),
                                axis=mybir.AxisListType.X,
                                op=ALU.add,
                            )
                            msgex = wp.tile([P, (FD + H) * G], BF16, tag="msgex")
                            W99 = FD + H
                            nc.scalar.activation(
                                out=v(msgex, FD, [msgex[:].ap[0], [W99, gc], [1, H]]),
                                in_=v(e_t, 0, [e_t[:].ap[0], [H, gc], [1, H]]),
                                func=AF.Exp,
                                bias=ebias[:],
                            )
                            nc.vector.tensor_tensor(
                                out=v(msgex, 0,
                                      [msgex[:].ap[0], [W99, gc], [HE, H], [1, HE]]),
                                in0=v(nins, FD,
                                      [nins[:].ap[0], [CW, gc], [HE, H], [1, HE]]),
                                in1=v(msgex, FD,
                                      [msgex[:].ap[0], [W99, gc], [1, H], [0, HE]]),
                                op=ALU.mult,
                            )
                            for j in range(gc):
                                nc.tensor.matmul(
                                    out=ps[:],
                                    lhsT=oh[:, j * P : (j + 1) * P],
                                    rhs=msgex[:, j * W99 : (j + 1) * W99],
                                    start=(c0 + j == 0),
                                    stop=(c0 + j == T - 1),
                                    skip_group_check=True,
                                )
                        # ---- block epilogue ----
                        sp = ep.tile([P, H], F32, tag="sp")
                        nc.vector.tensor_scalar_add(
                            out=sp[:], in0=ps[:, FD : FD + H], scalar1=EPS
                        )
                        rcp = ep.tile([P, H], F32, tag="rcp")
                        nc.vector.reciprocal(out=rcp[:], in_=sp[:])
                        rv = v(rcp, 0, [rcp[:].ap[0], [1, H], [0, HE]])
                        if l < 2:
                            hr = ep.tile([P, FD], BF16, tag="hr")
                            nc.scalar.activation(
                                out=hr[:], in_=ps[:, 0:FD], func=AF.Relu
                            )
                            ht = ep.tile([P, FD], BF16, tag="ht")
                            nc.vector.tensor_tensor(
                                out=v(ht, 0, [ht[:].ap[0], [HE, H], [1, HE]]),
                                in0=v(hr, 0, [hr[:].ap[0], [HE, H], [1, HE]]),
                                in1=rv,
                                op=ALU.mult,
                            )
                            trp2 = q_oht.tile([P, 2 * P * G], BF16, tag="trp")
                            htTp = trp2[0:FD, 0:P]
                            nc.tensor.transpose(
                                out=htTp, in_=ht[:], identity=ident[:]
                            )
                            nc.vector.tensor_copy(
                                out=nh_sb[0:FD, b * P : (b + 1) * P], in_=htTp
                            )
                            # tables for layer l+1, block b
                            ptn = q_ep.tile([P, CW + FD], F32, tag="ptn")
                            nc.tensor.matmul(
                                out=ptn[:, 0:CW],
                                lhsT=nh_sb[:, b * P : (b + 1) * P],
                                rhs=wcat_sb[l + 1][:],
                                start=True, stop=True,
                            )
                            nc.tensor.matmul(
                                out=ptn[:, CW : CW + FD],
                                lhsT=nh_sb[:, b * P : (b + 1) * P],
                                rhs=wnj_sb[l + 1][:],
                                start=True, stop=True,
                            )
                            cs = ep.tile([P, CW], BF16, tag="cs")
                            nc.scalar.activation(
                                out=cs[:], in_=ptn[:, 0:CW], func=AF.Copy
                            )
                            nc.sync.dma_start(
                                out=catL[b * P : (b + 1) * P, :], in_=cs[:]
                            )
                            nc.vector.tensor_copy(
                                out=njtab[:, b * FD : (b + 1) * FD],
                                in_=ptn[:, CW : CW + FD],
                            )
                        else:
                            htf = ep.tile([P, FD], F32, tag="htf")
                            nc.vector.tensor_tensor(
                                out=v(htf, 0, [htf[:].ap[0], [HE, H], [1, HE]]),
                                in0=v(ps, 0, [ps[:].ap[0], [HE, H], [1, HE]]),
                                in1=rv,
                                op=ALU.mult,
                            )
                            nc.sync.dma_start(
                                out=out3[b * P : (b + 1) * P, :], in_=htf[:]
                            )
                if l < 2:
                    if fake_collective:
                        # single-core timing stand-in: write-traffic of the
                        # AllGather receive (8x catL into catG slices)
                        for dd in range(c.ndev):
                            nc.sync.dma_start(
                                out=catG[dd * c.R : (dd + 1) * c.R, :],
                                in_=catL[:],
                            )
                    else:
                        nc.gpsimd.collective_compute(
                            "AllGather",
                            ALU.bypass,
                            replica_groups=rg,
                            ins=[catL[:]],
                            outs=[catG[:]],
                        )
                    if debug and l == 0:
                        cgd = pp.tile([P, CW], F32, tag="cgd")
                        for m in range(c.npad // P):
                            nc.sync.dma_start(
                                out=cgd[:],
                                in_=catG[m * P : (m + 1) * P, :],
                            )
                            nc.sync.dma_start(
                                out=dbg["d_cat"][m * P : (m + 1) * P, :],
                                in_=cgd[:],
                            )

    nc.compile()
    return nc


_CACHE = {}


def get_program(cfg, debug=False, fake_collective=False):
    key = (cfg.ndev, cfg.nblk, cfg.T, cfg.odf, cfg.g, debug, fake_collective)
    if key not in _CACHE:
        _CACHE[key] = build_program(cfg, debug=debug,
                                    fake_collective=fake_collective)
    return _CACHE[key]


def run(inputs, cfg, trace=False, debug=False):
    from concourse.bass_utils import run_bass_kernel_spmd

    percore = host_prep(inputs, cfg)
    nc = get_program(cfg, debug=debug)
    core_ids = list(range(cfg.ndev))
    res = run_bass_kernel_spmd(nc, percore, core_ids, trace=trace)
    outs = [res.results[i]["out3"] for i in range(cfg.ndev)]
    full = np.concatenate(outs, axis=0)
    return full, res


def make_cfg(inputs):
    src = np.asarray(inputs["src"]).astype(np.int64)
    dst = np.asarray(inputs["dst"]).astype(np.int64)
    n_real = 50000
    ndev = 8
    nblk = 49
    nblk_all = ndev * nblk
    lo = src < HALF
    bidx = dst // P
    cnt_lo = np.bincount(bidx[lo], minlength=nblk_all).reshape(ndev, nblk)
    cnt_hi = np.bincount(bidx[~lo], minlength=nblk_all).reshape(ndev, nblk)
    TL = np.ceil(cnt_lo.max(axis=0) / P).astype(int)
    TH = np.ceil(cnt_hi.max(axis=0) / P).astype(int)
    odf = np.asarray(inputs["countyodfeats"]).shape[1]
    return Cfg(ndev, nblk, TL, TH, odf, n_real)


def kernel(**inputs) -> np.ndarray:
    cfg = make_cfg(inputs)
    full, _ = run(inputs, cfg)
    idxs = np.asarray(inputs["idxs"]).astype(np.int64)
    return np.ascontiguousarray(full[idxs]).astype(np.float32)
